# revision 12
# baseline (speedup 1.0000x reference)
"""Diagonal SSM kernel (Vandermonde contraction) on 8 Trainium2 NeuronCores.

Math: K[d,h,l] = 2*Re( sum_n sc[d,h,n] * w[h,n]^l ),  l in [0, 2048)
  where w = exp(a*dt), sc = c * (exp(a*dt)-1)/a.

Sharding: d_model (H=1024) split contiguously, 128 channels per core.

Strategy (per core): split l = 128*c + j. The host precomputes fp16
tables in float64 so the device needs NO transcendentals. Channels are
processed in 32 tiles of 4 channels (2 complex pairs). Per tile the
device runs FOUR matmuls (K=64, M=32, F=128), one per channel, placed
on the four disjoint PE quadrants via tile_position:
  lhsT (stationary) = WT[h][2n+t, 16d+c] = {2Re,-2Im}(sigma),
        sigma = sc * w^(128c)  (coarse block rotation folded in)
  rhs  (moving)     = JT[h][2n+t, j] = {Re,Im}(w^j)
  out  = ps[32q+16d+c, j] = K[d, 4*tile+q, 128c+j]   (all 128 rows valid)
This folds the 16 coarse blocks into M instead of issuing 16 separate
F=128 matmuls per pair, cutting PE streaming 8x vs the c-outer scheme,
and ships only valid output rows (1 MiB/core vs 6.25).

DMA is the wall: every transfer uses >=512B partition lines (full rate),
batched into 8 input loads ([128, 2560B] each, JT+WT merged) on the SP
queue and 4 output stores ([128, 2048B]) on the ACT queue. PSUM is
evacuated f32->f16 on alternating ScalarE/VectorE.
"""
from contextlib import ExitStack

import numpy as np

import concourse.bass as bass
import concourse.bacc as bacc
import concourse.tile as tile
from concourse import mybir
from concourse.bass_utils import run_bass_kernel_spmd

N_CORES = 8
H = 1024          # d_model
N = 32            # d_state//2
D = 2             # directions
L = 2048          # sequence length
J = 128           # j-block
CBLK = L // J     # 16 coarse blocks (folded into matmul M)
HC = H // N_CORES     # 128 channels per core
NTILE = HC // 4       # 32 tiles of 4 channels
LG = 4                # tiles per load DMA  -> 8 loads
SG = 8                # tiles per store DMA -> 4 stores
TCOL = 2 * J + 2 * 64  # 384 f16 cols per tile: 2 pair bases + 2 wt blocks

_nc_cache = {}


def _build_nc(repeat: int = 1, hwloop: int = 1):
    """Build the Bass program. `repeat` unrolls the whole compute; `hwloop`
    wraps that in a hardware For_i loop (timing builds only) so one dispatch
    runs repeat*hwloop iterations with constant program size."""
    if (repeat, hwloop) in _nc_cache:
        return _nc_cache[(repeat, hwloop)]
    nc = bacc.Bacc("TRN2", target_bir_lowering=False, debug=False,
                   num_devices=N_CORES)
    f16 = mybir.dt.float16
    f32 = mybir.dt.float32

    in_d = nc.dram_tensor("jt", [NTILE // LG, 128, LG * TCOL], f16,
                          kind="ExternalInput")
    out_d = nc.dram_tensor("out", [NTILE // SG, 128, SG * J], f16,
                           kind="ExternalOutput")

    with tile.TileContext(nc) as tc:
        with ExitStack() as ctx:
            jt_pool = ctx.enter_context(tc.tile_pool(name="jt", bufs=8))
            st_pool = ctx.enter_context(tc.tile_pool(name="st", bufs=3))
            ps_pool = ctx.enter_context(
                tc.tile_pool(name="ps", bufs=8, space="PSUM"))

            def body():
                jts = []
                for g in range(NTILE // LG):
                    jt = jt_pool.tile([128, LG * TCOL], f16, tag="jt")
                    nc.sync.dma_start(jt[:], in_d.ap()[g])
                    jts.append(jt)
                for sg in range(NTILE // SG):
                    st = st_pool.tile([128, SG * J], f16, tag="st")
                    for i in range(SG):
                        t = sg * SG + i
                        g, o = divmod(t, LG)
                        o *= TCOL
                        jt = jts[g]
                        ps = ps_pool.tile([128, J], f32)
                        for pair in range(2):
                            # K=128 pair-stacked matmul: one F=128 stream
                            # serves both channels via block-diagonal weights
                            pb = o + pair * J            # basis cols
                            wb = o + 2 * J + pair * 64   # weight cols
                            nc.tensor.matmul(
                                ps[64 * pair:64 * pair + 64, :],
                                jt[:, wb:wb + 64],
                                jt[:, pb:pb + J],
                                start=True, stop=True,
                                tile_position=(0, 64 * pair),
                                skip_group_check=True,
                            )
                        # evac on alternating engines
                        if t % 2 == 1:
                            nc.scalar.copy(st[:, i * J:(i + 1) * J], ps[:])
                        else:
                            nc.vector.tensor_copy(
                                st[:, i * J:(i + 1) * J], ps[:])
                    nc.sync.dma_start(out_d.ap()[sg], st[:])

            if hwloop > 1:
                with tc.For_i(0, hwloop):
                    for _ in range(repeat):
                        body()
            else:
                for _ in range(repeat):
                    body()
    nc.compile()
    _nc_cache[(repeat, hwloop)] = nc
    return nc


def _host_tables(log_dt, log_a_real, a_imag, coeffs):
    """Per-core packed [NTILE//LG, 128, LG*TCOL] f16 tables (f64 math)."""
    dt = np.exp(log_dt.astype(np.float64))                       # [H]
    a = -np.exp(log_a_real.astype(np.float64)) + 1j * a_imag.astype(np.float64)
    da = a * dt[:, None]                                         # [H,N] c128
    c = coeffs[..., 0].astype(np.float64) + 1j * coeffs[..., 1].astype(np.float64)
    sc = c * (np.expm1(da) / a)[None]                            # [D,H,N]

    j = np.arange(J, dtype=np.float64)
    re = da.real[:, :, None] * j                                 # [H,N,J]
    im = da.imag[:, :, None] * j
    dec = np.exp(re)
    WjR = dec * np.cos(im)
    WjI = dec * np.sin(im)

    cs = np.arange(CBLK, dtype=np.float64)
    wJc = np.exp(da[:, :, None] * (J * cs))                      # [H,N,C]
    sig = sc[:, :, :, None] * wJc[None]                          # [D,H,N,C]

    ins = []
    for core in range(N_CORES):
        h0 = core * HC
        # basis rows 2n+t: [h, 2n+t, j]
        B = np.empty((HC, N, 2, J), np.float64)
        B[:, :, 0] = WjR[h0:h0 + HC]
        B[:, :, 1] = WjI[h0:h0 + HC]
        B = B.reshape(HC, 2 * N, J)
        # weights [h, 2n+t, 16d+c]
        s2 = sig[:, h0:h0 + HC].transpose(1, 2, 0, 3)            # [h,n,d,c]
        Wm = np.empty((HC, N, 2, D, CBLK), np.float64)
        Wm[:, :, 0] = 2.0 * s2.real
        Wm[:, :, 1] = -2.0 * s2.imag
        Wm = Wm.reshape(HC, 2 * N, D * CBLK)
        # pack tiles: [tau, 128, 384]; wt blocks are block-diagonal per pair
        Bq = B.reshape(NTILE, 4, 2 * N, J)
        Wq = Wm.reshape(NTILE, 4, 2 * N, D * CBLK)
        tb = np.zeros((NTILE, 128, TCOL), np.float64)
        tb[:, 0:64, 0:J] = Bq[:, 0]
        tb[:, 64:128, 0:J] = Bq[:, 1]
        tb[:, 0:64, J:2 * J] = Bq[:, 2]
        tb[:, 64:128, J:2 * J] = Bq[:, 3]
        tb[:, 0:64, 2 * J:2 * J + 32] = Wq[:, 0]
        tb[:, 64:128, 2 * J + 32:2 * J + 64] = Wq[:, 1]
        tb[:, 0:64, 2 * J + 64:2 * J + 96] = Wq[:, 2]
        tb[:, 64:128, 2 * J + 96:] = Wq[:, 3]
        ins.append(np.ascontiguousarray(
            tb.reshape(NTILE // LG, LG, 128, TCOL)
              .transpose(0, 2, 1, 3)
              .reshape(NTILE // LG, 128, LG * TCOL)).astype(np.float16))
    return ins


def _gather(results):
    """Assemble [D, H, L] f32 from per-core device-native outs."""
    outs = []
    for c in range(N_CORES):
        o = results[c]["out"]
        if o.shape == (D, HC, L):          # emulate() path
            outs.append(o)
            continue
        # [sg, 64*pair+32*h2+16d+cb, i*128+jj] -> [d, (sg,i,pair,h2), (cb,jj)]
        o = o.astype(np.float32).reshape(NTILE // SG, 2, 2, D, CBLK, SG, J)
        o = o.transpose(3, 0, 5, 1, 2, 4, 6)       # [d, sg, i, pair, h2, cb, jj]
        outs.append(o.reshape(D, HC, L))
    return np.concatenate(outs, axis=1)


def kernel(log_dt, log_a_real, a_imag, coeffs, sequence_length, _repeat=1,
           _run=None):
    assert int(sequence_length) == L
    log_dt = np.asarray(log_dt)
    log_a_real = np.asarray(log_a_real)
    a_imag = np.asarray(a_imag)
    coeffs = np.asarray(coeffs)
    ins = _host_tables(log_dt, log_a_real, a_imag, coeffs)
    nc = _build_nc(_repeat)
    in_maps = [{"jt": ins[c]} for c in range(N_CORES)]
    run = _run or (lambda n, m: run_bass_kernel_spmd(
        n, m, core_ids=list(range(N_CORES)), trace=False).results)
    results = run(nc, in_maps)
    return _gather(results)


def emulate(log_dt, log_a_real, a_imag, coeffs, sequence_length):
    """Numpy emulation of the device program (fp16 tables, fp32 accum)."""
    assert int(sequence_length) == L
    ins = _host_tables(log_dt, log_a_real, a_imag, coeffs)
    results = []
    for core in range(N_CORES):
        tb = ins[core].astype(np.float32).reshape(
            NTILE // LG, 128, LG, TCOL).transpose(0, 2, 1, 3).reshape(
            NTILE, 128, TCOL)
        out = np.empty((D, HC, L), np.float32)
        for t in range(NTILE):
            for pair in range(2):
                basis = tb[t, :, pair * J:(pair + 1) * J]
                wt = tb[t, :, 2 * J + pair * 64:2 * J + pair * 64 + 64]
                pm = (wt.T @ basis).reshape(2, D, CBLK, J)  # [h2, d, cb, jj]
                for h2 in range(2):
                    for d in range(D):
                        out[d, 4 * t + 2 * pair + h2] = pm[h2, d].reshape(L)
        results.append({"out": out})
    return _gather(results)
